# revision 40
# baseline (speedup 1.0000x reference)
"""MultiHeadAttention Bass/Tile kernel for Trainium2, 8 NeuronCores.

Sharding: (batch, head-half) -> 8 cores, zero collectives.
  core c: batch b = c//2, head half hh = c%2 (8 heads, full 2048 queries).
  Every projection column/row is computed exactly once across the 8 cores
  (no K/V duplication); the two head-halves' out-projection partial sums
  are added on the host at gather time (y[b] = part0 + part1).

Everything runs in bf16 (fp8 measured too coarse for the 2e-2 gate: each
fp8-quantized tensor alone costs 2-4% of output error via max statistics).

Per-core dataflow, software-pipelined so projection work fills the PE idle
time of the ACT-paced attention stream:
  upfront: Q^T[e,q] (all four q-quarters), K^T first s-quarter.
  head 0: remaining K^T quarters and all V strips emitted just-in-time
      between its 16 ks iterations (the tile framework's semaphores keep
      the data dependencies honest; emission order shapes the overlap).
  per head h, per k-strip ks: S^T[k,q] = K_h^T.T @ Q_h^T (PSUM f32, two
      1024-wide q halves) -> exp on ACT (bf16, masked in-place by a DVE 2x
      tensor_tensor with 1-mask) or DVE int-bitcast fast-exp with the mask
      folded into the Pool multiply (tunable split) -> P strip (bf16)
      -> PV accumulated in [q, 64]+den[q] orientation: the P strip is the
      stationary operand so each strip costs 64+1 output rows, not 512.
  norm: one DVE reciprocal per head ([128,16] packed dens; q lives on
      partitions so normalization is a per-partition scalar multiply)
      -> OA[q,(h,d)] bf16 -> PE-transpose 128x128 blocks -> OA^T[e,q].
  tail: y_partial[q, e_out] = OA^T.T @ w_out^T, f32 out.
"""

import os
import sys

for _p in ("/opt/trn_rl_repo", "/root/.axon_site/_ro/trn_rl_repo"):
    if os.path.isdir(_p) and _p not in sys.path:
        sys.path.insert(0, _p)

from contextlib import ExitStack

import numpy as np
import ml_dtypes

import concourse.tile as tile
from concourse import bacc, mybir
from concourse.bass_utils import run_bass_kernel_spmd

B, S, D = 4, 2048, 1024
H, HD = 16, 64
HL = 8          # local heads per core
EC = HL * HD    # local embedding width (512)
QL = S          # full query rows per core
NCORES = 8

F32 = mybir.dt.float32
BF16 = mybir.dt.bfloat16
I32 = mybir.dt.int32
BF = ml_dtypes.bfloat16

LAM = 0.125
FE_C1 = 12102203.161561485  # 2**23 / ln 2
FE_C2 = 1065353216.0 - 486408.0

# every FE_EVERY-th exp tile takes the DVE+Pool fast-exp path; 0 disables
FE_EVERY = 3

_NC_CACHE = {}


def _build_kernel(tc, t_in, t_out, dbg=None):
    nc = tc.nc
    qa_d, ka_d, va_d, m_d = t_in["qT"], t_in["kT"], t_in["vT"], t_in["mT"]
    wq_d, wk_d, wv_d, wo_d = t_in["wqT"], t_in["wkT"], t_in["wvT"], t_in["woT"]
    idt_d = t_in["idt"]
    y = t_out["y"]

    def dchunks(dram):  # [D or EC, cols] -> [128, n, cols]
        return dram[:, :].rearrange("(c p) q -> p c q", p=128)

    with ExitStack() as ctx:
        persist = ctx.enter_context(tc.tile_pool(name="persist", bufs=1))
        QT8 = persist.tile([128, 4, QL], BF16)    # [e%128, hpair, q]
        KT8 = persist.tile([128, 4, S], BF16)     # [e%128, hpair, s]
        V = persist.tile([128, 16, HL, HD], BF16)  # [s%128, ks, h, d]
        ONEc = persist.tile([128, 1], BF16)
        IDT = persist.tile([128, 128], BF16)
        OAn = persist.tile([128, 16, HL, HD], BF16)  # [q%128, qt, h, d]

        nc.sync.dma_start(out=IDT, in_=idt_d[:, :])
        nc.vector.memset(ONEc, 1.0)

        MBp = ctx.enter_context(tc.tile_pool(name="mbp", bufs=1, side="right"))  # diag
        MB = MBp.tile([128, 16, QL], BF16)        # (1-mask^T) strips

        def emit_mb_dmas():
            for mq in range(4):
                nc.sync.dma_start(
                    out=MB[:, 4 * mq : 4 * (mq + 1), :],
                    in_=m_d[512 * mq : 512 * (mq + 1), :]
                    .rearrange("(s p) q -> p s q", p=128),
                )

        nev = [0]

        def evac(out_ap, in_ap):
            if nev[0] % 2 == 0:
                nc.scalar.activation(
                    out=out_ap, in_=in_ap,
                    func=mybir.ActivationFunctionType.Copy,
                )
            else:
                nc.vector.tensor_copy(out=out_ap, in_=in_ap)
            nev[0] += 1

        stQK_ctx = ExitStack()
        stQK = stQK_ctx.enter_context(
            tc.tile_pool(name="stageQK", bufs=1, side="right")
        )
        WQ = stQK.tile([128, 8, EC], BF16, tag="wq")
        nc.sync.dma_start(out=WQ[:, :, 0:128], in_=dchunks(wq_d)[:, :, 0:128])
        WK = stQK.tile([128, 8, EC], BF16, tag="wk")

        stV_ctx = ExitStack()
        stV = stV_ctx.enter_context(
            tc.tile_pool(name="stageV", bufs=1, side="right")
        )
        WV = stV.tile([128, 8, EC], BF16, tag="wv")
        nc.sync.dma_start(out=WV, in_=dchunks(wv_d))

        with ExitStack() as p1ctx:
            qk_ring = p1ctx.enter_context(tc.tile_pool(name="qkr", bufs=2))
            ebuf = p1ctx.enter_context(tc.tile_pool(name="eb", bufs=3))
            va_ring = p1ctx.enter_context(tc.tile_pool(name="var", bufs=3))
            ppool = p1ctx.enter_context(tc.tile_pool(name="pp", bufs=5))
            ipool = p1ctx.enter_context(tc.tile_pool(name="it", bufs=2))
            dpool = p1ctx.enter_context(tc.tile_pool(name="dr", bufs=2))
            ps = p1ctx.enter_context(tc.tile_pool(name="ps", bufs=5, space="PSUM"))
            psV = p1ctx.enter_context(
                tc.tile_pool(name="psV", bufs=1, space="PSUM")
            )

            # --- P0 unit emitters (interleaved as fillers) ---
            def qp_dma(qn):
                qs = qk_ring.tile([128, 8, 512], BF16, tag="a")
                nc.sync.dma_start(
                    out=qs, in_=dchunks(qa_d)[:, :, 512 * qn : 512 * (qn + 1)]
                )
                return qs

            def qp_block(qn, qs):  # Q^T q-cols [512qn, 512qn+512), all 4 g
                for g in range(4):
                    pq = ps.tile([128, 512], F32, tag="s")
                    for c in range(8):
                        nc.tensor.matmul(
                            pq,
                            WQ[:, c, 128 * g : 128 * (g + 1)],
                            qs[:, c, :],
                            start=(c == 0), stop=(c == 7),
                        )
                    evac(QT8[:, g, 512 * qn : 512 * (qn + 1)], pq)

            def kp_dma(sn):
                ks_ = qk_ring.tile([128, 8, 512], BF16, tag="a")
                nc.sync.dma_start(
                    out=ks_, in_=dchunks(ka_d)[:, :, 512 * sn : 512 * (sn + 1)]
                )
                return ks_

            def kp_block(sn, ks_):  # K^T s-cols [512sn, 512sn+512), all 4 g
                for g in range(4):
                    pk = ps.tile([128, 512], F32, tag="s")
                    for c in range(8):
                        nc.tensor.matmul(
                            pk,
                            WK[:, c, 128 * g : 128 * (g + 1)],
                            ks_[:, c, :],
                            start=(c == 0), stop=(c == 7),
                        )
                    evac(KT8[:, g, 512 * sn : 512 * (sn + 1)], pk)

            def vp_dma(st):
                vs = va_ring.tile([128, 8, 128], BF16, tag="v")
                nc.sync.dma_start(
                    out=vs, in_=dchunks(va_d)[:, :, 128 * st : 128 * (st + 1)]
                )
                return vs

            def vp_block(st, vs):  # V rows [128st, 128st+128)
                pvp = ps.tile([128, 512], F32, tag="s")
                for c in range(8):
                    nc.tensor.matmul(
                        pvp,
                        vs[:, c, :],
                        WV[:, c, :],
                        start=(c == 0), stop=(c == 7),
                    )
                evac(
                    V[:, st, :, :],
                    pvp[:, :].rearrange("p (h d) -> p h d", h=HL),
                )

            def emit_pv(pvd, pvn, h, ks, Pk):
                # PSUM start=True zeroes the whole 2KB bank (zero region), so
                # only the first matmul touching each bank may set it; the
                # other subtile regions accumulate onto pending-zero bytes.
                for qt in range(16):
                    lhsT = Pk[:, 128 * qt : 128 * (qt + 1)]
                    nc.tensor.matmul(
                        pvd[:, qt, :], lhsT, V[:, ks, h, :],
                        start=(ks == 0 and qt % 8 == 0), stop=(ks == 15),
                        skip_group_check=True,
                    )
                    nc.tensor.matmul(
                        pvn[:, qt : qt + 1], lhsT, ONEc,
                        start=(ks == 0 and qt == 0), stop=(ks == 15),
                        skip_group_check=True,
                    )

            def head(h, fillers):
                g, poff = h // 2, 64 * (h % 2)
                pvd = psV.tile([128, 16, HD], F32, tag="pvd")
                pvn = psV.tile([128, 16], F32, tag="pvn")
                pq = []
                for ks in range(16):
                    for f in fillers.get(ks, ()):
                        f()
                    Pk = ppool.tile([128, QL], BF16)
                    for qh in range(2):
                        pk_half = Pk[:, 1024 * qh : 1024 * (qh + 1)]
                        idx = (h * 16 + ks) * 2 + qh
                        fe = FE_EVERY and idx % FE_EVERY == FE_EVERY - 1
                        for qn in range(2):
                            sp = ps.tile([128, 512], F32, tag="s")
                            nc.tensor.matmul(
                                sp,
                                KT8[poff : poff + 64, g,
                                    128 * ks : 128 * (ks + 1)],
                                QT8[poff : poff + 64, g,
                                    1024 * qh + 512 * qn :
                                    1024 * qh + 512 * (qn + 1)],
                                start=True, stop=True,
                            )
                            sl = slice(512 * qn, 512 * (qn + 1))
                            msl = MB[:, ks, 1024 * qh + 512 * qn :
                                     1024 * qh + 512 * (qn + 1)]
                            if fe:
                                it = ipool.tile([128, 512], I32)
                                nc.vector.tensor_scalar(
                                    out=it, in0=sp,
                                    scalar1=LAM * FE_C1, scalar2=FE_C2,
                                    op0=mybir.AluOpType.mult,
                                    op1=mybir.AluOpType.add,
                                )
                                nc.gpsimd.tensor_tensor(
                                    out=pk_half[:, sl],
                                    in0=it.bitcast(F32), in1=msl,
                                    op=mybir.AluOpType.mult,
                                )
                            else:
                                eb = ebuf.tile([128, 512], BF16)
                                nc.scalar.activation(
                                    out=eb, in_=sp,
                                    func=mybir.ActivationFunctionType.Exp,
                                    scale=LAM,
                                )
                                nc.vector.tensor_tensor(
                                    out=pk_half[:, sl], in0=eb,
                                    in1=msl,
                                    op=mybir.AluOpType.mult,
                                )
                    pq.append((ks, Pk))
                    if len(pq) > 4:
                        k2, p2 = pq.pop(0)
                        emit_pv(pvd, pvn, h, k2, p2)
                for k2, p2 in pq:
                    emit_pv(pvd, pvn, h, k2, p2)

                denr = dpool.tile([128, 16], F32)
                nc.vector.reciprocal(out=denr, in_=pvn)
                for qt in range(16):
                    if qt % 2 == 0:
                        nc.vector.tensor_scalar(
                            out=OAn[:, qt, h, :], in0=pvd[:, qt, :],
                            scalar1=denr[:, qt : qt + 1], scalar2=None,
                            op0=mybir.AluOpType.mult,
                        )
                    else:
                        nc.scalar.activation(
                            out=OAn[:, qt, h, :], in_=pvd[:, qt, :],
                            func=mybir.ActivationFunctionType.Copy,
                            scale=denr[:, qt : qt + 1],
                        )

            def transpose_qt(qt, psT):  # all 4 e-chunks for one q block
                pt4 = psT.tile([128, 4, 128], BF16, tag="t")
                for hp in range(4):
                    nc.tensor.matmul(
                        pt4[:, hp, :],
                        OAn[:, qt, 2 * hp : 2 * hp + 2, :]
                        .rearrange("p h d -> p (h d)"),
                        IDT,
                        start=(hp == 0), stop=(hp == 3), is_transpose=True,
                        skip_group_check=True,
                    )
                nc.vector.tensor_copy(
                    out=OAT[:, :, 128 * qt : 128 * (qt + 1)], in_=pt4
                )

            # ---- emission schedule ----
            qs0 = qk_ring.tile([128, 8, 512], BF16, tag="a")
            nc.sync.dma_start(
                out=qs0[:, 0:4, :], in_=dchunks(qa_d)[:, 0:4, 0:512]
            )
            nc.sync.dma_start(
                out=qs0[:, 4:8, :], in_=dchunks(qa_d)[:, 4:8, 0:512]
            )
            nc.sync.dma_start(
                out=WQ[:, :, 128:EC], in_=dchunks(wq_d)[:, :, 128:EC]
            )
            nc.sync.dma_start(out=WK, in_=dchunks(wk_d))
            qs_list = [qs0, qp_dma(1)]
            for qn in range(4):
                if qn + 2 < 4:
                    qs_list.append(qp_dma(qn + 2))
                qp_block(qn, qs_list[qn])
            ks0 = kp_dma(0)
            kp_block(0, ks0)
            emit_mb_dmas()

            # h0 fillers: dma 2 slots ahead of compute
            f0 = {ks: [] for ks in range(16)}
            kd = {}
            vd = {}
            for j in (1, 2, 3):
                f0[max(4 * j - 3, 0)].append(
                    lambda j=j: kd.__setitem__(j, kp_dma(j)))
                f0[4 * j - 1].append(lambda j=j: kp_block(j, kd[j]))
            vd[0] = vp_dma(0)
            vd[1] = vp_dma(1)
            for st in range(16):
                if st + 2 < 16:
                    f0[st].append(
                        lambda st=st: vd.__setitem__(st + 2, vp_dma(st + 2)))
                f0[st].append(lambda st=st: vp_block(st, vd[st]))

            head(0, f0)
            stV_ctx.close()
            stQK_ctx.close()
            p23 = ctx.enter_context(
                tc.tile_pool(name="p23", bufs=1, side="right")
            )
            OAT = p23.tile([128, 4, QL], BF16)  # OA^T [e%128, echunk, q]
            WO = p23.tile([128, 4, D], BF16, tag="wo")
            nc.sync.dma_start(out=WO, in_=dchunks(wo_d))

            for h in range(1, HL):
                head(h, {})

        if dbg is not None:
            nc.sync.dma_start(
                out=dbg["oan"][:, :],
                in_=OAn[:, :, :, :].rearrange("p a b c -> p (a b c)"),
            )
        # ---- tail: transposes + out projection, pipelined per q block ----
        with (
            tc.tile_pool(name="yb", bufs=8) as ybuf,
            tc.tile_pool(name="psT2", bufs=4, space="PSUM") as psT2,
            tc.tile_pool(name="psY", bufs=4, space="PSUM") as psY,
        ):
            transpose_qt(0, psT2)
            for qt in range(16):
                if qt + 1 < 16:
                    transpose_qt(qt + 1, psT2)
                for en in range(2):
                    psy = psY.tile([128, 512], F32)
                    for c in range(4):
                        nc.tensor.matmul(
                            psy,
                            OAT[:, c, 128 * qt : 128 * (qt + 1)],
                            WO[:, c, 512 * en : 512 * (en + 1)],
                            start=(c == 0), stop=(c == 3),
                        )
                    yb = ybuf.tile([128, 512], F32)
                    evac(yb, psy)
                    nc.sync.dma_start(
                        out=y[128 * qt : 128 * (qt + 1),
                              512 * en : 512 * (en + 1)],
                        in_=yb,
                    )
            if dbg is not None:
                nc.sync.dma_start(
                    out=dbg["oat"][:, :],
                    in_=OAT[:, :, :].rearrange("p a c -> p (a c)"),
                )


def _get_nc(debug=False):
    key = ("nc", debug)
    if key in _NC_CACHE:
        return _NC_CACHE[key]
    nc = bacc.Bacc("TRN2", target_bir_lowering=False)
    t_in = {
        "qT": nc.dram_tensor("qT", [D, QL], BF16, kind="ExternalInput"),
        "kT": nc.dram_tensor("kT", [D, S], BF16, kind="ExternalInput"),
        "vT": nc.dram_tensor("vT", [D, S], BF16, kind="ExternalInput"),
        "mT": nc.dram_tensor("mT", [S, QL], BF16, kind="ExternalInput"),
        "wqT": nc.dram_tensor("wqT", [D, EC], BF16, kind="ExternalInput"),
        "wkT": nc.dram_tensor("wkT", [D, EC], BF16, kind="ExternalInput"),
        "wvT": nc.dram_tensor("wvT", [D, EC], BF16, kind="ExternalInput"),
        "woT": nc.dram_tensor("woT", [EC, D], BF16, kind="ExternalInput"),
        "idt": nc.dram_tensor("idt", [128, 128], BF16, kind="ExternalInput"),
    }
    t_out = {"y": nc.dram_tensor("y", [QL, D], F32, kind="ExternalOutput")}
    dbg = None
    if debug:
        dbg = {
            "oan": nc.dram_tensor("oan", [128, 16 * HL * HD], BF16,
                                  kind="ExternalOutput"),
            "oat": nc.dram_tensor("oat", [128, 4 * QL], BF16,
                                  kind="ExternalOutput"),
        }
    with tile.TileContext(nc) as tc:
        _build_kernel(tc, t_in, t_out, dbg=dbg)
    nc.compile()
    _NC_CACHE[key] = nc
    return nc


def _bf(x):
    return np.asarray(x, np.float32).astype(BF)


def _in_maps(inputs):
    q = np.asarray(inputs["query"], np.float32)
    k = np.asarray(inputs["key"], np.float32)
    v = np.asarray(inputs["value"], np.float32)
    mask = np.asarray(inputs["mask"], np.int32)
    wqT = np.asarray(inputs["wq"], np.float32).T
    wkT = np.asarray(inputs["wk"], np.float32).T
    wvT = np.asarray(inputs["wv"], np.float32).T
    woT = np.asarray(inputs["w_out"], np.float32).T
    idt = np.eye(128, dtype=np.float32).astype(BF)
    maps = []
    for c in range(NCORES):
        b, hh = c // 2, c % 2
        esl = slice(hh * EC, (hh + 1) * EC)
        maps.append(
            {
                "qT": _bf(np.ascontiguousarray(q[b].T)),
                "kT": _bf(np.ascontiguousarray(k[b].T)),
                "vT": _bf(np.ascontiguousarray(v[b].T)),
                "mT": _bf(1.0 - np.ascontiguousarray(mask[b].T)),
                "wqT": _bf(np.ascontiguousarray(wqT[:, esl])),
                "wkT": _bf(np.ascontiguousarray(wkT[:, esl])),
                "wvT": _bf(np.ascontiguousarray(wvT[:, esl])),
                "woT": _bf(np.ascontiguousarray(woT[esl, :])),
                "idt": idt,
            }
        )
    return maps


def _gather(res):
    return np.stack(
        [
            res.results[2 * b]["y"] + res.results[2 * b + 1]["y"]
            for b in range(B)
        ]
    )


def kernel(**inputs) -> np.ndarray:
    nc = _get_nc()
    res = run_bass_kernel_spmd(nc, _in_maps(inputs), core_ids=list(range(NCORES)))
    return _gather(res)


def kernel_traced(**inputs):
    """Like kernel() but with NTFF tracing; returns (output, BassKernelResults)."""
    nc = _get_nc()
    res = run_bass_kernel_spmd(
        nc, _in_maps(inputs), core_ids=list(range(NCORES)), trace=True
    )
    return _gather(res), res
